# revision 2
# baseline (speedup 1.0000x reference)
"""minLSTM model kernel for Trainium2, data-parallel over batch on 8 cores.

Model (T=1024, B=64, IN=256, LSTM=1024, FNN=4096, OUT=256):
  zf/zi/zh = x @ Wf/Wi/Wh + b       3 gate GEMMs
  f,i = sigmoid(zf), sigmoid(zi);  a = f/(f+i);  bv = (i/(f+i))*zh
  h_t = a_t h_{t-1} + bv_t          linear recurrence over T (h_0 = 0)
  out = gelu_exact(h @ W1 + b1) @ W2 + b2

Mapping (per core, 8 batch elements):
  - everything runs in a "transposed" layout [channels, time] so the gate
    GEMM outputs feed the DVE tensor_tensor_scan (which scans along the
    free axis) and then feed the FFN GEMMs as the moving operand directly.
  - x is transposed + bf16-cast on the HOST, so no on-chip transposes for
    the gate GEMMs.  The final GEMM (h@W1->gelu->@W2) is computed with the
    gelu activations as the *stationary* operand, producing the output in
    natural [time, channel] layout - no output transpose either.
  - matmuls in bf16 (fp32 PSUM accumulation), gate elementwise in fp32,
    1/(f+i) via the fast custom-DVE reciprocal (18 bits, s is in [0.1, 2]).
  - exact erf-based gelu: g' = (x+b1)*(1+erf((x+b1)/sqrt2)) = 2*gelu, with
    the factor 0.5 folded into W2 on the host.
  - recurrence: state = (a * state) - d1 with d1 = (a-1)*zh_biased, i.e.
    the hardware scan op with op0=mult, op1=subtract.
"""

import numpy as np
import ml_dtypes

import concourse.mybir as mybir
import concourse.tile as tile
from concourse import bacc
from concourse.bass_utils import run_bass_kernel_spmd

F32 = mybir.dt.float32
BF16 = mybir.dt.bfloat16
AF = mybir.ActivationFunctionType
ALU = mybir.AluOpType

T, B, IN, LSTM, FNN, OUT = 1024, 64, 256, 1024, 4096, 256
NCORES = 8
BL = B // NCORES          # batch per core
KI = IN // 128            # 2   K-tiles of the gate GEMMs
ML = LSTM // 128          # 8   M-tiles of LSTM  (also K-tiles of GEMM2)
MF = FNN // 128           # 32  M-tiles of FNN   (also K-tiles of GEMM3)
HALF = 512                # time columns processed per superblock
NSB = T // HALF           # halves per batch element

_CACHE = {}


def _build():
    nc = bacc.Bacc("TRN2", target_bir_lowering=False, debug=False)

    xT = nc.dram_tensor("xT", [BL, KI, 128, T], BF16, kind="ExternalInput").ap()
    wf = nc.dram_tensor("wf", [KI, 128, LSTM], BF16, kind="ExternalInput").ap()
    wi = nc.dram_tensor("wi", [KI, 128, LSTM], BF16, kind="ExternalInput").ap()
    wh = nc.dram_tensor("wh", [KI, 128, LSTM], BF16, kind="ExternalInput").ap()
    w1 = nc.dram_tensor("w1", [ML, 128, FNN], BF16, kind="ExternalInput").ap()
    w2 = nc.dram_tensor("w2", [MF, 128, OUT], BF16, kind="ExternalInput").ap()
    bfi = nc.dram_tensor("bfi", [128, ML], F32, kind="ExternalInput").ap()
    bii = nc.dram_tensor("bii", [128, ML], F32, kind="ExternalInput").ap()
    bhi = nc.dram_tensor("bhi", [128, ML], F32, kind="ExternalInput").ap()
    b1i = nc.dram_tensor("b1i", [128, MF], F32, kind="ExternalInput").ap()
    b1s = nc.dram_tensor("b1s", [128, MF], F32, kind="ExternalInput").ap()
    b2b = nc.dram_tensor("b2b", [128, OUT], F32, kind="ExternalInput").ap()
    out = nc.dram_tensor("out", [T, BL, OUT], F32, kind="ExternalOutput").ap()

    with tile.TileContext(nc) as tc:
        with (
            tc.tile_pool(name="const", bufs=1) as cons,
            tc.tile_pool(name="io", bufs=1) as io,
            tc.tile_pool(name="work", bufs=1) as wk,
            tc.tile_pool(name="psg", bufs=1, space="PSUM") as psg,
            tc.tile_pool(name="ps1", bufs=2, space="PSUM") as ps1p,
            tc.tile_pool(name="ps3", bufs=2, space="PSUM") as ps3p,
        ):
            # ---- resident weights/biases -------------------------------
            wf_sb = [cons.tile([128, LSTM], BF16, tag=f"wf{k}", name=f"wf{k}")
                     for k in range(KI)]
            wi_sb = [cons.tile([128, LSTM], BF16, tag=f"wi{k}", name=f"wi{k}")
                     for k in range(KI)]
            wh_sb = [cons.tile([128, LSTM], BF16, tag=f"wh{k}", name=f"wh{k}")
                     for k in range(KI)]
            w1_sb = [cons.tile([128, FNN], BF16, tag=f"w1_{k}", name=f"w1_{k}")
                     for k in range(ML)]
            w2_sb = [cons.tile([128, OUT], BF16, tag=f"w2_{k}", name=f"w2_{k}")
                     for k in range(MF)]
            for k in range(KI):
                nc.sync.dma_start(out=wf_sb[k], in_=wf[k])
                nc.sync.dma_start(out=wi_sb[k], in_=wi[k])
                nc.sync.dma_start(out=wh_sb[k], in_=wh[k])
            for k in range(ML):
                nc.sync.dma_start(out=w1_sb[k], in_=w1[k])
            for k in range(MF):
                nc.sync.dma_start(out=w2_sb[k], in_=w2[k])
            bf_sb = cons.tile([128, ML], F32, tag="bf", name="bf")
            bi_sb = cons.tile([128, ML], F32, tag="bi", name="bi")
            bh_sb = cons.tile([128, ML], F32, tag="bh", name="bh")
            b1_sb = cons.tile([128, MF], F32, tag="b1", name="b1")
            b1s_sb = cons.tile([128, MF], F32, tag="b1s", name="b1s")
            b2_sb = cons.tile([128, OUT], F32, tag="b2", name="b2")
            nc.sync.dma_start(out=bf_sb, in_=bfi)
            nc.sync.dma_start(out=bi_sb, in_=bii)
            nc.sync.dma_start(out=bh_sb, in_=bhi)
            nc.sync.dma_start(out=b1_sb, in_=b1i)
            nc.sync.dma_start(out=b1s_sb, in_=b1s)
            nc.sync.dma_start(out=b2_sb, in_=b2b)

            # ---- main loop: software-pipelined superblocks -------------
            # emit_gates(sb) runs two superblocks ahead of emit_ffn(sb) so
            # the ACT+DVE gate/scan chain drains while PE crunches the FFN
            # GEMMs of earlier superblocks (PE never waits on the scan).
            h_sb = {}      # sb -> list of 8 h tiles

            def emit_gates(sb):
                b, hf = divmod(sb, NSB)
                c0 = hf * HALF
                xt = [io.tile([128, HALF], BF16, tag=f"xt{k}", bufs=3,
                              name=f"xt{k}_{sb}") for k in range(KI)]
                for k in range(KI):
                    nc.sync.dma_start(out=xt[k], in_=xT[b, k][:, c0:c0 + HALF])
                h_cur = [None] * ML
                for m in range(ML):
                    ms = slice(m * 128, (m + 1) * 128)
                    psf = psg.tile([128, HALF], F32, tag="psf",
                                   name=f"psf_{sb}_{m}")
                    psi = psg.tile([128, HALF], F32, tag="psi",
                                   name=f"psi_{sb}_{m}")
                    psh = psg.tile([128, HALF], F32, tag="psh",
                                   name=f"psh_{sb}_{m}")
                    for k in range(KI):
                        nc.tensor.matmul(psf, lhsT=wf_sb[k][:, ms], rhs=xt[k],
                                         start=(k == 0), stop=(k == KI - 1))
                    for k in range(KI):
                        nc.tensor.matmul(psi, lhsT=wi_sb[k][:, ms], rhs=xt[k],
                                         start=(k == 0), stop=(k == KI - 1))
                    for k in range(KI):
                        nc.tensor.matmul(psh, lhsT=wh_sb[k][:, ms], rhs=xt[k],
                                         start=(k == 0), stop=(k == KI - 1))
                    f = wk.tile([128, HALF], F32, tag="f", bufs=3,
                                name=f"f_{sb}_{m}")
                    fi = wk.tile([128, HALF], F32, tag="fi", bufs=3,
                                 name=f"fi_{sb}_{m}")
                    ht = wk.tile([128, HALF], BF16, tag="ht", bufs=3,
                                 name=f"ht_{sb}_{m}")
                    nc.scalar.activation(f, psf, AF.Sigmoid,
                                         bias=bf_sb[:, m:m + 1])
                    nc.scalar.activation(fi, psi, AF.Sigmoid,
                                         bias=bi_sb[:, m:m + 1])
                    nc.scalar.add(ht, psh, bh_sb[:, m:m + 1])
                    s = wk.tile([128, HALF], F32, tag="s", bufs=3,
                                name=f"s_{sb}_{m}")
                    nc.vector.tensor_add(s, f, fi)
                    r = wk.tile([128, HALF], F32, tag="r", bufs=3,
                                name=f"r_{sb}_{m}")
                    nc.vector.reciprocal_approx_fast(r, s)
                    a = wk.tile([128, HALF], BF16, tag="a", bufs=3,
                                name=f"a_{sb}_{m}")
                    nc.vector.tensor_mul(a, f, r)
                    d1 = wk.tile([128, HALF], BF16, tag="d1", bufs=3,
                                 name=f"d1_{sb}_{m}")
                    nc.vector.scalar_tensor_tensor(
                        d1, in0=a, scalar=1.0, in1=ht,
                        op0=ALU.subtract, op1=ALU.mult)
                    h = wk.tile([128, HALF], BF16, tag=f"h{m}", bufs=3,
                                name=f"h{m}_{sb}")
                    init = (0.0 if hf == 0 else
                            h_sb[sb - 1][m][:, HALF - 1:HALF])
                    nc.vector.tensor_tensor_scan(
                        h, a, d1, init, op0=ALU.mult, op1=ALU.subtract)
                    h_cur[m] = h
                h_sb[sb] = h_cur

            def emit_ffn(sb):
                b, hf = divmod(sb, NSB)
                c0 = hf * HALF
                h_cur = h_sb[sb]
                g_cur = [None] * MF
                for fm in range(MF):
                    fs = slice(fm * 128, (fm + 1) * 128)
                    ps1 = ps1p.tile([128, HALF], F32, tag="ps1",
                                    name=f"ps1_{sb}_{fm}")
                    for k in range(ML):
                        nc.tensor.matmul(ps1, lhsT=w1_sb[k][:, fs],
                                         rhs=h_cur[k], start=(k == 0),
                                         stop=(k == ML - 1))
                    e = wk.tile([128, HALF], BF16, tag="e", bufs=3,
                                name=f"e_{sb}_{fm}")
                    nc.scalar.activation(e, ps1, AF.Erf,
                                         bias=b1s_sb[:, fm:fm + 1],
                                         scale=0.7071067811865476)
                    xb = wk.tile([128, HALF], BF16, tag="xb", bufs=3,
                                 name=f"xb_{sb}_{fm}")
                    nc.scalar.add(xb, ps1, b1_sb[:, fm:fm + 1])
                    g = wk.tile([128, HALF], BF16, tag=f"g{fm}", bufs=1,
                                name=f"g{fm}_{sb}")
                    nc.vector.scalar_tensor_tensor(
                        g, in0=e, scalar=1.0, in1=xb,
                        op0=ALU.add, op1=ALU.mult)
                    g_cur[fm] = g
                for tch in range(HALF // 128):
                    ts_ = slice(tch * 128, (tch + 1) * 128)
                    ps3 = ps3p.tile([128, OUT], F32, tag="ps3",
                                    name=f"ps3_{sb}_{tch}")
                    for k in range(MF):
                        nc.tensor.matmul(ps3, lhsT=g_cur[k][:, ts_],
                                         rhs=w2_sb[k], start=(k == 0),
                                         stop=(k == MF - 1))
                    ot = wk.tile([128, OUT], F32, tag="ot", bufs=3,
                                 name=f"ot_{sb}_{tch}")
                    nc.vector.tensor_add(ot, ps3, b2_sb)
                    t0 = c0 + tch * 128
                    nc.sync.dma_start(out=out[t0:t0 + 128, b, :], in_=ot)

            NTOT = BL * NSB
            emit_gates(0)
            emit_gates(1)
            for sb in range(NTOT):
                emit_ffn(sb)
                if sb + 2 < NTOT:
                    emit_gates(sb + 2)
                h_sb.pop(sb - 1, None)   # drop refs no longer needed
    nc.compile()
    return nc


def _prep_inputs(x, Wf, bf, Wi, bi, Wh, bh, W1, b1, W2, b2):
    bfc = lambda v: np.ascontiguousarray(v).astype(ml_dtypes.bfloat16)
    # x: [T, B, IN] -> per-core [BL, KI, 128, T] bf16, transposed on host
    xt = np.ascontiguousarray(x.transpose(1, 2, 0))          # [B, IN, T]
    xt = bfc(xt).reshape(NCORES, BL, KI, 128, T)
    wf_t = bfc(Wf).reshape(KI, 128, LSTM)
    wi_t = bfc(Wi).reshape(KI, 128, LSTM)
    wh_t = bfc(Wh).reshape(KI, 128, LSTM)
    w1_t = bfc(W1).reshape(ML, 128, FNN)
    w2_t = bfc(W2.astype(np.float64) * 0.5).reshape(MF, 128, OUT)
    col = lambda v, m: np.ascontiguousarray(
        v.astype(np.float32).reshape(m, 128).T)              # [128, m]
    shared = {
        "wf": wf_t, "wi": wi_t, "wh": wh_t, "w1": w1_t, "w2": w2_t,
        "bfi": col(bf, ML), "bii": col(bi, ML), "bhi": col(bh, ML),
        "b1i": col(b1, MF),
        "b1s": col(b1.astype(np.float64) / np.sqrt(2.0), MF),
        "b2b": np.ascontiguousarray(
            np.broadcast_to(b2.astype(np.float32), (128, OUT))),
    }
    return [dict(shared, xT=np.ascontiguousarray(xt[c])) for c in range(NCORES)]


def run(inputs, trace=False):
    if "nc" not in _CACHE:
        _CACHE["nc"] = _build()
    nc = _CACHE["nc"]
    in_maps = _prep_inputs(**inputs)
    res = run_bass_kernel_spmd(nc, in_maps, list(range(NCORES)), trace=trace)
    out = np.concatenate([res.results[c]["out"] for c in range(NCORES)],
                         axis=1)
    return out, res


def kernel(**inputs):
    out, _ = run(inputs, trace=False)
    return out
